# revision 25
# baseline (speedup 1.0000x reference)
"""Trainium2 Bass kernel for nn_BatchMegaDecode (32-layer hyena/attention hybrid,
single decode step), v2.

Strategy: 8-way tensor parallel on one trn2 chip.
- proj_W / mlp_W1 column-sharded (by attention-head / hyena-channel groups),
  out_W / mlp_W2 row-sharded; partial outputs exchanged via ncfw AllGather
  (Shared-space DRAM bounce) + local reduce; a dummy warm-up collective at
  kernel start hides the ~50us first-call ncfw latency.
- Single-decode-step algebra folded on the host:
  * rmsnorm weights fold into the following GEMM's columns; the rsqrt scalar
    is applied AFTER the GEMM (linearity), so the rsq chain overlaps the GEMM.
  * FIR tap-2 scaling folds into proj rows; FIR state dot-products (read-only
    this step) fold into per-channel constants; all three hyena block types
    collapse to y2 = (a*x1v + E)*x2 with host constants a, E.
- KV caches sharded by head; batch replicated.
"""

import sys
import types
import numpy as np
import ml_dtypes

BF = ml_dtypes.bfloat16

for _p in ("/opt/trn_rl_repo",):
    if _p not in sys.path:
        sys.path.append(_p)

import concourse.bass as bass
import concourse.bacc as bacc
import concourse.tile as tile
import concourse.mybir as mybir
from concourse import bass_utils

L, H, NH, HD, F, B, S = 32, 1024, 16, 64, 2048, 2, 2048
HPH = H // NH
EPS = 1e-6
NC = 8
f32 = mybir.dt.float32
bf16 = mybir.dt.bfloat16
AF = mybir.ActivationFunctionType
ALU = mybir.AluOpType
AX = mybir.AxisListType
SCALE = HD ** -0.5

BLOCK = {0: 'HCS', 4: 'HCS', 7: 'HCS', 11: 'HCS', 14: 'HCS', 18: 'HCS', 21: 'HCS', 25: 'HCS', 28: 'HCS',
         1: 'HCM', 5: 'HCM', 8: 'HCM', 12: 'HCM', 15: 'HCM', 19: 'HCM', 22: 'HCM', 26: 'HCM', 29: 'HCM',
         2: 'HCL', 6: 'HCL', 9: 'HCL', 13: 'HCL', 16: 'HCL', 20: 'HCL', 23: 'HCL', 27: 'HCL', 30: 'HCL',
         3: 'ATT', 10: 'ATT', 17: 'ATT', 24: 'ATT', 31: 'ATT'}

# small pack layout (f32 cols)
SW_C = 0      # [6]  zp bias C, (m, b): non-ATT: w2*pb + st0*w0 + st1*w1 + sfb; ATT: pb repl.
SW_A = 6      # [1]  per-channel mixer multiplier a
SW_E = 7      # [2]  per-channel-batch mixer constant E
SW_OB = 9     # [16] out_b replicated over batch, (t, b)
SMALL_W = 25


def _prep_core_inputs(inputs, c, pos):
    """Build the per-core numpy input dict for core c."""
    d = {}
    heads = slice(2 * c, 2 * c + 2)
    ch = slice(128 * c, 128 * c + 128)
    fsl = slice(256 * c, 256 * c + 256)

    x = np.asarray(inputs['x'], np.float32)
    d['xT'] = np.ascontiguousarray(x[:, 0, :].T.reshape(8, 128, 2).transpose(1, 0, 2))

    att_idx = 0
    nt = pos // 128          # full 128-tiles of cached context
    rem = pos % 128

    for i in range(L):
        t = BLOCK[i]
        small = np.zeros((128, SMALL_W), np.float32)
        n1 = np.asarray(inputs['norm1_w'][i], np.float32)   # [H]
        n2 = np.asarray(inputs['norm2_w'][i], np.float32)

        if t == 'ATT':
            pr = np.asarray(inputs['proj_W'][i], np.float32).reshape(3, NH, HD, H)[:, heads]
            pw_c = pr.reshape(384, H) * n1[None, :]
            pb = np.asarray(inputs['proj_b'][i], np.float32).reshape(3, NH, HD)[:, heads].reshape(3, 128)
            small[:, SW_C + 0:SW_C + 6:2] = pb.T       # C[m, b=0]
            small[:, SW_C + 1:SW_C + 6:2] = pb.T       # C[m, b=1]
        else:
            pr = np.asarray(inputs['proj_W'][i], np.float32).reshape(NH, 3, HPH, H)[heads]
            pw_c = pr.transpose(1, 0, 2, 3).reshape(384, H) * n1[None, :]
            pb = np.asarray(inputs['proj_b'][i], np.float32).reshape(NH, 3, HPH)[heads].transpose(1, 0, 2).reshape(3, 128)
            sfw = np.asarray(inputs['sf_w'][i], np.float32).reshape(NH, 3, HPH, 3)[heads].transpose(1, 0, 2, 3).reshape(3, 128, 3)
            sfb = np.asarray(inputs['sf_b'][i], np.float32).reshape(NH, 3, HPH)[heads].transpose(1, 0, 2).reshape(3, 128)
            fs = np.asarray(inputs['fir_state'][i], np.float32).reshape(B, NH, 3, HPH, 2)[:, heads]
            fs = fs.transpose(2, 1, 3, 0, 4).reshape(3, 128, B, 2)  # [m, 128, b, cache]
            # fold tap2 into proj rows; C = w2*pb + st0*w0 + st1*w1 + sfb
            w2 = sfw[:, :, 2]                           # [3, 128]
            pw_c = pw_c * w2.reshape(384, 1)
            Cmb = (w2 * pb + sfb)[:, :, None] + np.einsum('mpbc,mpc->mpb', fs, sfw[:, :, :2])
            small[:, SW_C:SW_C + 6] = Cmb.transpose(1, 0, 2).reshape(128, 6)

        d[f'pwT_{i}'] = np.ascontiguousarray(pw_c.T.reshape(8, 128, 384).transpose(1, 0, 2)).astype(BF)
        ob = np.asarray(inputs['out_b'][i], np.float32).reshape(8, 128).T  # [128, 8]
        small[:, SW_OB:SW_OB + 16] = np.repeat(ob, 2, axis=1) / NC

        if t == 'HCS':
            h7 = np.asarray(inputs['hcs_h'][i], np.float32)[ch]           # [128, 7]
            D = np.asarray(inputs['hcs_D'][i], np.float32)[ch]            # [128]
            st = np.asarray(inputs['hcs_state'][i], np.float32)[:, ch]    # [B, 128, 6]
            small[:, SW_A] = h7[:, 6]
            small[:, SW_E:SW_E + 2] = (st * h7[None, :, :6]).sum(-1).T + D[:, None]
        elif t == 'HCM':
            h128 = np.asarray(inputs['hcm_h'][i], np.float32)[ch]         # [128, 128]
            D = np.asarray(inputs['hcm_D'][i], np.float32)[ch]
            st = np.asarray(inputs['hcm_state'][i], np.float32)[:, ch]    # [B, 128, 127]
            wr = h128[:, ::-1]
            small[:, SW_A] = wr[:, 127] + D
            small[:, SW_E:SW_E + 2] = (st * wr[None, :, :127]).sum(-1).T
        elif t == 'HCL':
            poles = np.exp(np.asarray(inputs['hcl_logpoles'][i], np.float32)[ch])  # [128, 16]
            res = np.asarray(inputs['hcl_residues'][i], np.float32)[ch]
            D = np.asarray(inputs['hcl_D'][i], np.float32)[ch]
            st = np.asarray(inputs['iir_state'][i], np.float32)[:, ch]    # [B, 128, 16]
            small[:, SW_A] = res.sum(-1) + D
            small[:, SW_E:SW_E + 2] = (st * (res * poles)[None]).sum(-1).T
        else:  # ATT
            kc = np.asarray(inputs['k_cache'][att_idx], np.float32)[:, :pos, heads]   # [B, pos, 2, 64]
            # fold the 1/sqrt(HD) scale into cached K
            kT = kc.transpose(2, 3, 0, 1).reshape(128, B, pos) * SCALE   # [(h,d), b, s]
            ktp = np.zeros((128, B, (nt + (1 if rem else 0)) * 128), np.float32)
            ktp[:, :, :pos] = kT
            d[f'kT_{att_idx}'] = np.ascontiguousarray(ktp.reshape(128, -1)).astype(BF)
            vc = np.asarray(inputs['v_cache'][att_idx], np.float32)[:, :pos, heads]   # [B, pos, 2, 64]
            nt2 = nt + (1 if rem else 0)
            vh = np.zeros((128, nt2, B, 128), np.float32)  # [s_row, t, b, (h,d)]
            vfull = vc[:, :nt * 128].reshape(B, nt, 128, 128).transpose(2, 1, 0, 3)
            vh[:, :nt] = vfull
            if rem:
                vh[:rem, nt] = vc[:, nt * 128:pos].transpose(1, 0, 2, 3).reshape(rem, B, 128)
            d[f'v_{att_idx}'] = np.ascontiguousarray(vh.reshape(128, -1)).astype(BF)
            att_idx += 1

        d[f'small_{i}'] = small
        wo = np.asarray(inputs['out_W'][i], np.float32)[:, ch]            # [1024, 128]
        d[f'owT_{i}'] = np.ascontiguousarray(wo.T).astype(BF)             # [128, 1024]
        w1 = np.asarray(inputs['mlp_W1'][i], np.float32)[fsl] * n2[None, :]  # [256, 1024]
        d[f'm1T_{i}'] = np.ascontiguousarray(w1.reshape(256, 8, 128).transpose(2, 1, 0)).astype(BF)
        w2m = np.asarray(inputs['mlp_W2'][i], np.float32)[:, fsl]         # [1024, 256]
        d[f'm2T_{i}'] = np.ascontiguousarray(w2m.T.reshape(2, 128, 1024).transpose(1, 0, 2)).astype(BF)

    # constants
    cos_t = np.asarray(inputs['rope_cos'], np.float32)[pos]  # [32]
    sin_t = np.asarray(inputs['rope_sin'], np.float32)[pos]
    c64 = np.concatenate([cos_t, cos_t])
    s64 = np.concatenate([sin_t, sin_t])
    ssign = np.where(np.arange(64) < 32, -s64, s64)
    # unscaled rope constants (scale folded into kT / exp)
    ropec = np.stack([np.tile(c64, 2), np.tile(ssign, 2)], axis=1)  # [128, 2]
    d['ropec'] = np.ascontiguousarray(ropec.astype(np.float32))
    d['ones128'] = np.ones((128, 1), np.float32).astype(BF)
    d['ones1'] = np.ones((1, 128), np.float32)
    return d


N_LAYERS = L


def _build(pos, n_layers=None):
    n_layers = N_LAYERS if n_layers is None else n_layers
    nt = pos // 128
    rem = pos % 128
    nt2 = nt + (1 if rem else 0)

    nc = bacc.Bacc("TRN2", target_bir_lowering=False, debug=False, num_devices=NC)

    din = {}
    def dram_in(name, shape, dt=f32):
        din[name] = nc.dram_tensor(name, list(shape), dt, kind="ExternalInput")
        return din[name]

    dram_in('xT', [128, 8, 2])
    att_idx = 0
    for i in range(L):
        dram_in(f'pwT_{i}', [128, 8, 384], bf16)
        dram_in(f'small_{i}', [128, SMALL_W])
        dram_in(f'owT_{i}', [128, 1024], bf16)
        dram_in(f'm1T_{i}', [128, 8, 256], bf16)
        dram_in(f'm2T_{i}', [128, 2, 1024], bf16)
        if BLOCK[i] == 'ATT':
            dram_in(f'kT_{att_idx}', [128, 2 * nt2 * 128], bf16)
            dram_in(f'v_{att_idx}', [128, nt2 * 2 * 128], bf16)
            att_idx += 1
    for nme, shp, dt_ in [('ropec', [128, 2], f32), ('ones128', [128, 1], bf16),
                          ('ones1', [1, 128], f32)]:
        dram_in(nme, shp, dt_)
    out_t = nc.dram_tensor('out', [2, 1024], f32, kind="ExternalOutput")

    # ---- remote-dma junction machinery ----
    fD = nc.alloc_semaphore("fakeD")
    fL = nc.alloc_semaphore("fakeL")
    rsems = [nc.alloc_semaphore(f"jrsem{i}") for i in range(2)]
    lsem = nc.alloc_semaphore("jlsem")
    rdests = [(0, k) for k in range(NC)]
    patches = []
    jj_counter = [0]

    with tile.TileContext(nc) as tc:
        with tc.tile_pool(name="wts", bufs=3) as wp, \
             tc.tile_pool(name="wk", bufs=2) as wk, \
             tc.tile_pool(name="att", bufs=2) as ap_, \
             tc.tile_pool(name="cst", bufs=1) as cp, \
             tc.tile_pool(name="ps", bufs=1, space="PSUM") as pp, \
             tc.tile_pool(name="dram", bufs=3, space="DRAM") as dp:

            # dummy collective: pays the ~50us first-call ncfw warmup while the
            # initial weight DMAs and layer-0 compute proceed. Input filled by a
            # DRAM->DRAM copy from an existing input so the doorbell fires ~4us in.
            win = dp.tile([128, 1], f32, tag="win")
            nc.sync.dma_start(out=win[:], in_=din['ropec'][:, 0:1])
            wout = dp.tile([1024, 1], f32, tag="wout", addr_space="Shared")
            nc.gpsimd.collective_compute(
                "AllGather", ALU.bypass,
                replica_groups=[list(range(NC))],
                ins=[win.opt()], outs=[wout.opt()],
            )

            # persistent consts
            ropec = cp.tile([128, 2], f32, tag="ropec")
            ones128 = cp.tile([128, 1], bf16, tag="ones128")
            ones1 = cp.tile([1, 128], f32, tag="ones1")
            ones1b = cp.tile([1, 128], bf16, tag="ones1b")
            for t_, n_ in [(ropec, 'ropec'), (ones128, 'ones128'), (ones1, 'ones1')]:
                nc.sync.dma_start(out=t_[:], in_=din[n_][:, :])
            nc.vector.tensor_copy(ones1b[:], ones1[:])
            eps_t = cp.tile([1, 1], f32, tag="eps")
            nc.vector.memset(eps_t[:], EPS)
            qbd = cp.tile([128, 2, 2], bf16, tag="qbd")
            nc.vector.memset(qbd[:], 0.0)

            x = wk.tile([128, 8, 2], f32, tag="x")
            nc.sync.dma_start(out=x[:], in_=din['xT'][:, :, :])
            xb = wk.tile([128, 8, 2], bf16, tag="xb")
            nc.vector.tensor_copy(xb[:], x[:])

            def rsq_chain(x_t, tagp):
                """returns rsb [128, 2] f32 = 1/sqrt(mean(x^2)+eps) broadcast to partitions.
                rsqrt computed on DVE (bit-trick seed + 2 Newton steps) to keep the
                scalar engine's activation table pinned to Gelu/Exp."""
                xsq = wk.tile([128, 8, 2], bf16, tag=f"xsq{tagp}")
                nc.vector.tensor_mul(xsq[:], x_t[:], x_t[:])
                ms = wk.tile([128, 2], bf16, tag=f"ms{tagp}")
                with nc.allow_low_precision(reason="bf16 sumsq is plenty for rmsnorm"):
                    nc.vector.tensor_reduce(ms[:], xsq[:].rearrange("p t b -> p b t"),
                                            axis=AX.X, op=ALU.add)
                pss = pp.tile([1, 2], f32, tag="misc")
                nc.tensor.matmul(pss[:], ones128[:], ms[:], start=True, stop=True)
                # m_half = 0.5*(mean+eps); seed magic pre-adjusted for the 0.5x
                # so one Newton step lands at ~0.17% max rel err
                m_t = wk.tile([1, 2], f32, name=f"m_t{tagp}", tag=f"m_t{tagp}")
                nc.vector.tensor_scalar(m_t[:], pss[:], 0.5 / H, 0.5 * EPS,
                                        op0=ALU.mult, op1=ALU.add)
                mi = m_t[:].bitcast(mybir.dt.int32)
                yi_t = wk.tile([1, 2], mybir.dt.int32, name=f"yi{tagp}", tag=f"yi{tagp}")
                nc.vector.tensor_scalar(yi_t[:], mi, 1, None, op0=ALU.logical_shift_right)
                yw = wk.tile([1, 2], f32, name=f"yw{tagp}", tag=f"yw{tagp}")
                t_t = wk.tile([1, 2], f32, name=f"tt{tagp}", tag=f"tt{tagp}")
                nc.vector.tensor_scalar(yw[:].bitcast(mybir.dt.int32), yi_t[:], -1, 0x5EF7520F,
                                        op0=ALU.mult, op1=ALU.add)
                nc.vector.tensor_mul(t_t[:], yw[:], yw[:])
                nc.vector.tensor_mul(t_t[:], t_t[:], m_t[:])
                nc.vector.tensor_scalar(t_t[:], t_t[:], -1.0, 1.5,
                                        op0=ALU.mult, op1=ALU.add)
                yb = wk.tile([1, 2], bf16, name=f"yb{tagp}", tag=f"yb{tagp}")
                nc.vector.tensor_mul(yb[:], yw[:], t_t[:])
                prsb = pp.tile([128, 2], f32, name=f"prsb{tagp}", tag=f"prsb{tagp}")
                nc.tensor.matmul(prsb[:], ones1b[:], yb[:], start=True, stop=True)
                return prsb

            def junction(stage_src_psum, ob_t, resid_t):
                """all-gather partials via ncfw collective; returns x_new [128,8,2] f32.
                stage_src_psum: psum tile [128, 8, 2] partial; ob_t: [128,16] bias or None
                """
                st = wk.tile([128, 16], f32, tag="stg")
                if ob_t is not None:
                    nc.vector.tensor_add(st[:], stage_src_psum[:].rearrange("p t b -> p (t b)"), ob_t)
                else:
                    nc.vector.tensor_copy(st[:], stage_src_psum[:].rearrange("p t b -> p (t b)"))
                jin = dp.tile([128, 16], f32, tag="jin")
                nc.sync.dma_start(out=jin[:], in_=st[:])
                jout = dp.tile([1024, 16], f32, tag="jout", addr_space="Shared")
                nc.gpsimd.collective_compute(
                    "AllGather", ALU.bypass,
                    replica_groups=[list(range(NC))],
                    ins=[jin.opt()], outs=[jout.opt()],
                    unique_tensors="Yes",
                )
                land = wk.tile([128, 8, 16], f32, tag="land")
                nc.sync.dma_start(out=land[:], in_=jout[:, :].rearrange("(r p) f -> p r f", p=128))
                red = wk.tile([128, 16], f32, tag="red")
                nc.vector.tensor_reduce(red[:], land[:].rearrange("p r f -> p f r"),
                                        axis=AX.X, op=ALU.add)
                nx = wk.tile([128, 8, 2], f32, tag="x")
                nc.vector.tensor_add(nx[:].rearrange("p t b -> p (t b)"), red[:],
                                     resid_t[:].rearrange("p t b -> p (t b)"))
                return nx

            att_idx = 0
            for i in range(n_layers):
                bt = BLOCK[i]
                pwT = wp.tile([128, 8, 384], bf16, tag="pwT")
                nc.gpsimd.dma_start(out=pwT[:], in_=din[f'pwT_{i}'][:, :, :])
                small = wp.tile([128, SMALL_W], f32, tag="small")
                nc.gpsimd.dma_start(out=small[:], in_=din[f'small_{i}'][:, :])
                owT = wp.tile([128, 1024], bf16, tag="owT")
                nc.gpsimd.dma_start(out=owT[:], in_=din[f'owT_{i}'][:, :])
                m1T = wp.tile([128, 8, 256], bf16, tag="m1T")
                nc.gpsimd.dma_start(out=m1T[:], in_=din[f'm1T_{i}'][:, :, :])
                m2T = wp.tile([128, 2, 1024], bf16, tag="m2T")
                nc.gpsimd.dma_start(out=m2T[:], in_=din[f'm2T_{i}'][:, :, :])
                if bt == 'ATT':
                    kT = ap_.tile([128, 2, nt2 * 128], bf16, tag="kT")
                    nc.gpsimd.dma_start(out=kT[:], in_=din[f'kT_{att_idx}'][:, :].rearrange(
                        "p (b s) -> p b s", b=2))
                    vv = ap_.tile([128, nt2, 2, 128], bf16, tag="vv")
                    nc.gpsimd.dma_start(out=vv[:], in_=din[f'v_{att_idx}'][:, :].rearrange(
                        "p (t b hd) -> p t b hd", t=nt2, b=2))

                # proj GEMV (n1 + tap2 folded); rsq chain runs in parallel
                pz = pp.tile([128, 3, 2], f32, tag="zh")
                for m in range(3):
                    for kt in range(8):
                        nc.tensor.matmul(pz[:, m, :], pwT[:, kt, m * 128:(m + 1) * 128],
                                         xb[:, kt, :], start=(kt == 0), stop=(kt == 7))
                rsb = rsq_chain(x, "1")

                # zp = pz * rsq + C  (deferred rmsnorm + FIR-1 with host constants)
                zp = wk.tile([128, 3, 2], f32, tag="zp")
                for b in range(2):
                    nc.vector.scalar_tensor_tensor(
                        zp[:, :, b], pz[:, :, b], rsb[:, b:b + 1],
                        small[:, SW_C + b:SW_C + 6:2],
                        op0=ALU.mult, op1=ALU.add)

                if bt != 'ATT':
                    x1v = wk.tile([128, 2], f32, tag="x1v")
                    nc.vector.tensor_mul(x1v[:], zp[:, 1, :], zp[:, 2, :])
                    ytmp = wk.tile([128, 2], f32, tag="ytmp")
                    nc.vector.scalar_tensor_tensor(
                        ytmp[:], x1v[:], small[:, SW_A:SW_A + 1], small[:, SW_E:SW_E + 2],
                        op0=ALU.mult, op1=ALU.add)
                    y2 = wk.tile([128, 2], bf16, tag="y2")
                    nc.vector.tensor_mul(y2[:], ytmp[:], zp[:, 0, :])
                else:
                    # ---- attention ----
                    # rope on q,k jointly: zp[:, 0:2, :] (cols (m=q/k, b))
                    scrA = wk.tile([1, 1], f32, tag="scrA")
                    nc.scalar.activation(scrA[:], zp[0:1, 0, 0:1], AF.Exp)
                    rtmp = wk.tile([128, 2, 2], f32, tag="rtmp")
                    for base in (0, 64):
                        nc.vector.tensor_copy(rtmp[base:base + 32, :, :], zp[base + 32:base + 64, 0:2, :])
                        nc.vector.tensor_copy(rtmp[base + 32:base + 64, :, :], zp[base:base + 32, 0:2, :])
                    qk = wk.tile([128, 2, 2], f32, tag="qk")
                    nc.vector.tensor_scalar_mul(rtmp[:], rtmp[:], ropec[:, 1:2])
                    nc.vector.scalar_tensor_tensor(qk[:], zp[:, 0:2, :], ropec[:, 0:1],
                                                   rtmp[:], op0=ALU.mult, op1=ALU.add)
                    # qbd: block-diag q (bf16); kr: rope'd k
                    nc.vector.tensor_copy(qbd[0:64, :, 0], qk[0:64, 0, :])
                    nc.vector.tensor_copy(qbd[64:128, :, 1], qk[64:128, 0, :])
                    kr = wk.tile([128, 2], bf16, tag="kr")
                    nc.vector.tensor_copy(kr[:], qk[:, 1, :])
                    v_sb = wk.tile([128, 2], f32, tag="v_sb")
                    nc.vector.tensor_copy(v_sb[:], zp[:, 2, :])

                    # scores (transposed): psc [128(s), t, (b,h)]
                    psc = pp.tile([128, nt2, 4], f32, tag="psc")
                    for t_ in range(nt2):
                        pr = 128 if (t_ < nt or rem == 0) else rem
                        if pr < 128:
                            nc.vector.memset(psc[:, t_, :], 0.0)
                        for b in range(2):
                            nc.tensor.matmul(psc[0:pr, t_, 2 * b:2 * b + 2],
                                             kT[:, b, t_ * 128:t_ * 128 + pr],
                                             qbd[:, b, :], start=True, stop=True)
                    # current-token score [1, (b,h)]
                    pcur = pp.tile([1, 2, 2], f32, tag="apsum")
                    for b in range(2):
                        nc.tensor.matmul(pcur[:, b, :], kr[:, b:b + 1], qbd[:, b, :],
                                         start=True, stop=True)
                    # exp
                    esc = wk.tile([128, nt2, 4], bf16, tag="esc")
                    nc.scalar.activation(esc[:], psc[:], AF.Exp)
                    ecur = wk.tile([1, 4], f32, tag="ecur")
                    nc.scalar.activation(ecur[:], pcur[:].rearrange("p b h -> p (b h)"), AF.Exp, scale=SCALE)
                    scrB = wk.tile([1, 1], f32, tag="scrB")
                    nc.scalar.activation(scrB[:], ecur[0:1, 0:1], AF.Gelu_apprx_tanh)
                    # sums over s: ones-matmul -> [1, t*4] -> reduce over t -> [1,4]
                    pse = pp.tile([1, nt2 * 4], f32, name="pse", tag="apsum")
                    nc.tensor.matmul(pse[:], ones128[:], esc[:].rearrange("p t c -> p (t c)"),
                                     start=True, stop=True)
                    sev = wk.tile([1, 8], f32, tag="sev")
                    nc.vector.tensor_reduce(sev[:, 0:4], pse[:].rearrange("p (t c) -> p c t", c=4),
                                            axis=AX.X, op=ALU.add)
                    nc.vector.tensor_add(sev[:, 0:4], sev[:, 0:4], ecur[:])
                    nc.vector.reciprocal(sev[:, 0:4], sev[:, 0:4])
                    nc.vector.tensor_copy(sev[:, 4:8], ecur[:])
                    # broadcast (rec, ecur) to all partitions
                    prcb = pp.tile([128, 8], f32, name="prcb", tag="apsum")
                    nc.tensor.matmul(prcb[:], ones1[:], sev[:], start=True, stop=True)
                    # ctx: py [128, b, hw] accumulated over tiles
                    py = pp.tile([128, 2, 2], f32, tag="py")
                    for b in range(2):
                        for t_ in range(nt2):
                            pr = 128 if (t_ < nt or rem == 0) else rem
                            nc.tensor.matmul(py[:, b, :], vv[0:pr, t_, b, :],
                                             esc[0:pr, t_, 2 * b:2 * b + 2],
                                             start=(t_ == 0), stop=(t_ == nt2 - 1))
                    # y2 = (py_diag + v*ecur_b) * rec_b  per head-half
                    vc_t = wk.tile([128, 2], f32, tag="vc_t")
                    tsum = wk.tile([128, 2], f32, tag="tsum")
                    y2 = wk.tile([128, 2], bf16, tag="y2")
                    for hb, sl in ((0, slice(0, 64)), (1, slice(64, 128))):
                        nc.vector.tensor_mul(vc_t[sl, :], v_sb[sl, :],
                                             prcb[sl, 4 + hb:8:2])
                        nc.vector.tensor_add(tsum[sl, :], py[sl, :, hb], vc_t[sl, :])
                        nc.vector.tensor_mul(y2[sl, :], tsum[sl, :], prcb[sl, hb:4:2])

                # out proj: partial^T [128, 8(t), 2(b)]
                pp1 = pp.tile([128, 8, 2], f32, tag="pj")
                for m in range(8):
                    nc.tensor.matmul(pp1[:, m, :], owT[:, m * 128:(m + 1) * 128], y2[:],
                                     start=True, stop=True)
                x_mid = junction(pp1, small[:, SW_OB:SW_OB + 16], x)

                # mlp (n2 folded into m1T cols; rsq2 applied post-GEMM)
                xmb = wk.tile([128, 8, 2], bf16, tag="xmb")
                nc.vector.tensor_copy(xmb[:], x_mid[:])
                ph = pp.tile([128, 2, 2], f32, name="ph", tag="zh")
                for m in range(2):
                    for kt in range(8):
                        nc.tensor.matmul(ph[:, m, :], m1T[:, kt, m * 128:(m + 1) * 128],
                                         xmb[:, kt, :], start=(kt == 0), stop=(kt == 7))
                rsb2 = rsq_chain(x_mid, "2")
                hz = wk.tile([128, 2, 2], f32, tag="hz")
                for b in range(2):
                    nc.vector.tensor_scalar_mul(hz[:, :, b], ph[:, :, b], rsb2[:, b:b + 1])
                hg = wk.tile([128, 2, 2], bf16, tag="hg")
                nc.scalar.activation(hg[:], hz[:], AF.Gelu_apprx_tanh)
                pp2 = pp.tile([128, 8, 2], f32, name="pp2", tag="pj")
                for m in range(8):
                    for kt in range(2):
                        nc.tensor.matmul(pp2[:, m, :], m2T[:, kt, m * 128:(m + 1) * 128],
                                         hg[:, kt, :], start=(kt == 0), stop=(kt == 1))
                x = junction(pp2, None, x_mid)
                xb = wk.tile([128, 8, 2], bf16, tag="xb")
                nc.vector.tensor_copy(xb[:], x[:])

                if bt == 'ATT':
                    att_idx += 1

            for b in range(2):
                nc.sync.dma_start(out=out_t.ap()[b].rearrange("(t p) -> p t", p=128),
                                  in_=x[:, :, b])

    # patch fake waits -> real remote sems
    for iname, fakenum, snum, sval, sname in patches:
        inst = nc.inst_map[iname]
        ow = inst.sync_info.on_wait
        hits = [w for w in ow if w.id == fakenum]
        assert len(hits) == 1, (iname, [str(w) for w in ow])
        hits[0].id = snum
        hits[0].wait_value = sval
        hits[0].ant_name = sname

    nc.compile()
    return nc


_CACHE = {}


def kernel(**inputs):
    pos = int(np.asarray(inputs['position']))
    if pos not in _CACHE:
        _CACHE[pos] = _build(pos)
    nc = _CACHE[pos]
    in_maps = [_prep_core_inputs(inputs, c, pos) for c in range(NC)]
    res = bass_utils.run_bass_kernel_spmd(nc, in_maps, core_ids=list(range(NC)))
    out = res.results[0]['out']  # [2, 1024], replicated across cores
    return out.reshape(B, 1, H).astype(np.float32)


# revision 26
# speedup vs baseline: 1.0160x; 1.0160x over previous
"""Trainium2 Bass kernel for nn_BatchMegaDecode (32-layer hyena/attention hybrid,
single decode step), v2.

Strategy: 8-way tensor parallel on one trn2 chip.
- proj_W / mlp_W1 column-sharded (by attention-head / hyena-channel groups),
  out_W / mlp_W2 row-sharded; partial outputs exchanged via ncfw AllGather
  (Shared-space DRAM bounce) + local reduce; a dummy warm-up collective at
  kernel start hides the ~50us first-call ncfw latency.
- Single-decode-step algebra folded on the host:
  * rmsnorm weights fold into the following GEMM's columns; the rsqrt scalar
    is applied AFTER the GEMM (linearity), so the rsq chain overlaps the GEMM.
  * FIR tap-2 scaling folds into proj rows; FIR state dot-products (read-only
    this step) fold into per-channel constants; all three hyena block types
    collapse to y2 = (a*x1v + E)*x2 with host constants a, E.
- KV caches sharded by head; batch replicated.
"""

import sys
import types
import numpy as np
import ml_dtypes

BF = ml_dtypes.bfloat16

for _p in ("/opt/trn_rl_repo",):
    if _p not in sys.path:
        sys.path.append(_p)

import concourse.bass as bass
import concourse.bacc as bacc
import concourse.tile as tile
import concourse.mybir as mybir
from concourse import bass_utils

L, H, NH, HD, F, B, S = 32, 1024, 16, 64, 2048, 2, 2048
HPH = H // NH
EPS = 1e-6
NC = 8
f32 = mybir.dt.float32
bf16 = mybir.dt.bfloat16
AF = mybir.ActivationFunctionType
ALU = mybir.AluOpType
AX = mybir.AxisListType
SCALE = HD ** -0.5

BLOCK = {0: 'HCS', 4: 'HCS', 7: 'HCS', 11: 'HCS', 14: 'HCS', 18: 'HCS', 21: 'HCS', 25: 'HCS', 28: 'HCS',
         1: 'HCM', 5: 'HCM', 8: 'HCM', 12: 'HCM', 15: 'HCM', 19: 'HCM', 22: 'HCM', 26: 'HCM', 29: 'HCM',
         2: 'HCL', 6: 'HCL', 9: 'HCL', 13: 'HCL', 16: 'HCL', 20: 'HCL', 23: 'HCL', 27: 'HCL', 30: 'HCL',
         3: 'ATT', 10: 'ATT', 17: 'ATT', 24: 'ATT', 31: 'ATT'}

# small pack layout (f32 cols)
SW_C = 0      # [6]  zp bias C, (m, b): non-ATT: w2*pb + st0*w0 + st1*w1 + sfb; ATT: pb repl.
SW_A = 6      # [1]  per-channel mixer multiplier a
SW_E = 7      # [2]  per-channel-batch mixer constant E
SW_OB = 9     # [16] out_b replicated over batch, (t, b)
SMALL_W = 25


def _prep_core_inputs(inputs, c, pos):
    """Build the per-core numpy input dict for core c."""
    d = {}
    heads = slice(2 * c, 2 * c + 2)
    ch = slice(128 * c, 128 * c + 128)
    fsl = slice(256 * c, 256 * c + 256)

    x = np.asarray(inputs['x'], np.float32)
    d['xT'] = np.ascontiguousarray(x[:, 0, :].T.reshape(8, 128, 2).transpose(1, 0, 2))

    att_idx = 0
    nt = pos // 128          # full 128-tiles of cached context
    rem = pos % 128

    for i in range(L):
        t = BLOCK[i]
        small = np.zeros((128, SMALL_W), np.float32)
        n1 = np.asarray(inputs['norm1_w'][i], np.float32)   # [H]
        n2 = np.asarray(inputs['norm2_w'][i], np.float32)

        if t == 'ATT':
            pr = np.asarray(inputs['proj_W'][i], np.float32).reshape(3, NH, HD, H)[:, heads]
            pw_c = pr.reshape(384, H) * n1[None, :]
            pb = np.asarray(inputs['proj_b'][i], np.float32).reshape(3, NH, HD)[:, heads].reshape(3, 128)
            small[:, SW_C + 0:SW_C + 6:2] = pb.T       # C[m, b=0]
            small[:, SW_C + 1:SW_C + 6:2] = pb.T       # C[m, b=1]
        else:
            pr = np.asarray(inputs['proj_W'][i], np.float32).reshape(NH, 3, HPH, H)[heads]
            pw_c = pr.transpose(1, 0, 2, 3).reshape(384, H) * n1[None, :]
            pb = np.asarray(inputs['proj_b'][i], np.float32).reshape(NH, 3, HPH)[heads].transpose(1, 0, 2).reshape(3, 128)
            sfw = np.asarray(inputs['sf_w'][i], np.float32).reshape(NH, 3, HPH, 3)[heads].transpose(1, 0, 2, 3).reshape(3, 128, 3)
            sfb = np.asarray(inputs['sf_b'][i], np.float32).reshape(NH, 3, HPH)[heads].transpose(1, 0, 2).reshape(3, 128)
            fs = np.asarray(inputs['fir_state'][i], np.float32).reshape(B, NH, 3, HPH, 2)[:, heads]
            fs = fs.transpose(2, 1, 3, 0, 4).reshape(3, 128, B, 2)  # [m, 128, b, cache]
            # fold tap2 into proj rows; C = w2*pb + st0*w0 + st1*w1 + sfb
            w2 = sfw[:, :, 2]                           # [3, 128]
            pw_c = pw_c * w2.reshape(384, 1)
            Cmb = (w2 * pb + sfb)[:, :, None] + np.einsum('mpbc,mpc->mpb', fs, sfw[:, :, :2])
            small[:, SW_C:SW_C + 6] = Cmb.transpose(1, 0, 2).reshape(128, 6)

        d[f'pwT_{i}'] = np.ascontiguousarray(pw_c.T.reshape(8, 128, 384).transpose(1, 0, 2)).astype(BF)
        ob = np.asarray(inputs['out_b'][i], np.float32).reshape(8, 128).T  # [128, 8]
        small[:, SW_OB:SW_OB + 16] = np.repeat(ob, 2, axis=1) / NC

        if t == 'HCS':
            h7 = np.asarray(inputs['hcs_h'][i], np.float32)[ch]           # [128, 7]
            D = np.asarray(inputs['hcs_D'][i], np.float32)[ch]            # [128]
            st = np.asarray(inputs['hcs_state'][i], np.float32)[:, ch]    # [B, 128, 6]
            small[:, SW_A] = h7[:, 6]
            small[:, SW_E:SW_E + 2] = (st * h7[None, :, :6]).sum(-1).T + D[:, None]
        elif t == 'HCM':
            h128 = np.asarray(inputs['hcm_h'][i], np.float32)[ch]         # [128, 128]
            D = np.asarray(inputs['hcm_D'][i], np.float32)[ch]
            st = np.asarray(inputs['hcm_state'][i], np.float32)[:, ch]    # [B, 128, 127]
            wr = h128[:, ::-1]
            small[:, SW_A] = wr[:, 127] + D
            small[:, SW_E:SW_E + 2] = (st * wr[None, :, :127]).sum(-1).T
        elif t == 'HCL':
            poles = np.exp(np.asarray(inputs['hcl_logpoles'][i], np.float32)[ch])  # [128, 16]
            res = np.asarray(inputs['hcl_residues'][i], np.float32)[ch]
            D = np.asarray(inputs['hcl_D'][i], np.float32)[ch]
            st = np.asarray(inputs['iir_state'][i], np.float32)[:, ch]    # [B, 128, 16]
            small[:, SW_A] = res.sum(-1) + D
            small[:, SW_E:SW_E + 2] = (st * (res * poles)[None]).sum(-1).T
        else:  # ATT
            kc = np.asarray(inputs['k_cache'][att_idx], np.float32)[:, :pos, heads]   # [B, pos, 2, 64]
            # fold the 1/sqrt(HD) scale into cached K
            kT = kc.transpose(2, 3, 0, 1).reshape(128, B, pos) * SCALE   # [(h,d), b, s]
            ktp = np.zeros((128, B, (nt + (1 if rem else 0)) * 128), np.float32)
            ktp[:, :, :pos] = kT
            d[f'kT_{att_idx}'] = np.ascontiguousarray(ktp.reshape(128, -1)).astype(BF)
            vc = np.asarray(inputs['v_cache'][att_idx], np.float32)[:, :pos, heads]   # [B, pos, 2, 64]
            nt2 = nt + (1 if rem else 0)
            vh = np.zeros((128, nt2, B, 128), np.float32)  # [s_row, t, b, (h,d)]
            vfull = vc[:, :nt * 128].reshape(B, nt, 128, 128).transpose(2, 1, 0, 3)
            vh[:, :nt] = vfull
            if rem:
                vh[:rem, nt] = vc[:, nt * 128:pos].transpose(1, 0, 2, 3).reshape(rem, B, 128)
            d[f'v_{att_idx}'] = np.ascontiguousarray(vh.reshape(128, -1)).astype(BF)
            att_idx += 1

        d[f'small_{i}'] = small
        wo = np.asarray(inputs['out_W'][i], np.float32)[:, ch]            # [1024, 128]
        d[f'owT_{i}'] = np.ascontiguousarray(wo.T).astype(BF)             # [128, 1024]
        w1 = np.asarray(inputs['mlp_W1'][i], np.float32)[fsl] * n2[None, :]  # [256, 1024]
        d[f'm1T_{i}'] = np.ascontiguousarray(w1.reshape(256, 8, 128).transpose(2, 1, 0)).astype(BF)
        w2m = np.asarray(inputs['mlp_W2'][i], np.float32)[:, fsl]         # [1024, 256]
        d[f'm2T_{i}'] = np.ascontiguousarray(w2m.T.reshape(2, 128, 1024).transpose(1, 0, 2)).astype(BF)

    # constants
    cos_t = np.asarray(inputs['rope_cos'], np.float32)[pos]  # [32]
    sin_t = np.asarray(inputs['rope_sin'], np.float32)[pos]
    c64 = np.concatenate([cos_t, cos_t])
    s64 = np.concatenate([sin_t, sin_t])
    ssign = np.where(np.arange(64) < 32, -s64, s64)
    # unscaled rope constants (scale folded into kT / exp)
    ropec = np.stack([np.tile(c64, 2), np.tile(ssign, 2)], axis=1)  # [128, 2]
    d['ropec'] = np.ascontiguousarray(ropec.astype(np.float32))
    d['ones128'] = np.ones((128, 1), np.float32).astype(BF)
    d['ones1'] = np.ones((1, 128), np.float32)
    return d


N_LAYERS = L


def _build(pos, n_layers=None):
    n_layers = N_LAYERS if n_layers is None else n_layers
    nt = pos // 128
    rem = pos % 128
    nt2 = nt + (1 if rem else 0)

    nc = bacc.Bacc("TRN2", target_bir_lowering=False, debug=False, num_devices=NC)

    din = {}
    def dram_in(name, shape, dt=f32):
        din[name] = nc.dram_tensor(name, list(shape), dt, kind="ExternalInput")
        return din[name]

    dram_in('xT', [128, 8, 2])
    att_idx = 0
    for i in range(L):
        dram_in(f'pwT_{i}', [128, 8, 384], bf16)
        dram_in(f'small_{i}', [128, SMALL_W])
        dram_in(f'owT_{i}', [128, 1024], bf16)
        dram_in(f'm1T_{i}', [128, 8, 256], bf16)
        dram_in(f'm2T_{i}', [128, 2, 1024], bf16)
        if BLOCK[i] == 'ATT':
            dram_in(f'kT_{att_idx}', [128, 2 * nt2 * 128], bf16)
            dram_in(f'v_{att_idx}', [128, nt2 * 2 * 128], bf16)
            att_idx += 1
    for nme, shp, dt_ in [('ropec', [128, 2], f32), ('ones128', [128, 1], bf16),
                          ('ones1', [1, 128], f32)]:
        dram_in(nme, shp, dt_)
    out_t = nc.dram_tensor('out', [2, 1024], f32, kind="ExternalOutput")

    # ---- remote-dma junction machinery ----
    fD = nc.alloc_semaphore("fakeD")
    fL = nc.alloc_semaphore("fakeL")
    rsems = [nc.alloc_semaphore(f"jrsem{i}") for i in range(2)]
    lsem = nc.alloc_semaphore("jlsem")
    rdests = [(0, k) for k in range(NC)]
    patches = []
    jj_counter = [0]

    with tile.TileContext(nc) as tc:
        with tc.tile_pool(name="wts", bufs=3) as wp, \
             tc.tile_pool(name="wk", bufs=2) as wk, \
             tc.tile_pool(name="att", bufs=2) as ap_, \
             tc.tile_pool(name="cst", bufs=1) as cp, \
             tc.tile_pool(name="ps", bufs=1, space="PSUM") as pp, \
             tc.tile_pool(name="dram", bufs=3, space="DRAM") as dp:

            # persistent consts
            ropec = cp.tile([128, 2], f32, tag="ropec")
            ones128 = cp.tile([128, 1], bf16, tag="ones128")
            ones1 = cp.tile([1, 128], f32, tag="ones1")
            ones1b = cp.tile([1, 128], bf16, tag="ones1b")
            for t_, n_ in [(ropec, 'ropec'), (ones128, 'ones128'), (ones1, 'ones1')]:
                nc.sync.dma_start(out=t_[:], in_=din[n_][:, :])
            nc.vector.tensor_copy(ones1b[:], ones1[:])
            eps_t = cp.tile([1, 1], f32, tag="eps")
            nc.vector.memset(eps_t[:], EPS)
            qbd = cp.tile([128, 2, 2], bf16, tag="qbd")
            nc.vector.memset(qbd[:], 0.0)

            # dummy collective: pays the ~50us first-call ncfw warmup while the
            # initial weight DMAs and layer-0 compute proceed
            wrm = wk.tile([128, 1], f32, name="wrm", tag="wrm")
            nc.vector.memset(wrm[:], 0.0)
            win = dp.tile([128, 1], f32, tag="win")
            nc.sync.dma_start(out=win[:], in_=wrm[:])
            wout = dp.tile([1024, 1], f32, tag="wout", addr_space="Shared")
            nc.gpsimd.collective_compute(
                "AllGather", ALU.bypass,
                replica_groups=[list(range(NC))],
                ins=[win.opt()], outs=[wout.opt()],
            )

            x = wk.tile([128, 8, 2], f32, tag="x")
            nc.sync.dma_start(out=x[:], in_=din['xT'][:, :, :])
            xb = wk.tile([128, 8, 2], bf16, tag="xb")
            nc.vector.tensor_copy(xb[:], x[:])

            def rsq_chain(x_t, tagp):
                """returns rsb [128, 2] f32 = 1/sqrt(mean(x^2)+eps) broadcast to partitions.
                rsqrt computed on DVE (bit-trick seed + 2 Newton steps) to keep the
                scalar engine's activation table pinned to Gelu/Exp."""
                xsq = wk.tile([128, 8, 2], bf16, tag=f"xsq{tagp}")
                nc.vector.tensor_mul(xsq[:], x_t[:], x_t[:])
                ms = wk.tile([128, 2], bf16, tag=f"ms{tagp}")
                with nc.allow_low_precision(reason="bf16 sumsq is plenty for rmsnorm"):
                    nc.vector.tensor_reduce(ms[:], xsq[:].rearrange("p t b -> p b t"),
                                            axis=AX.X, op=ALU.add)
                pss = pp.tile([1, 2], f32, tag="misc")
                nc.tensor.matmul(pss[:], ones128[:], ms[:], start=True, stop=True)
                # m_half = 0.5*(mean+eps); seed magic pre-adjusted for the 0.5x
                # so one Newton step lands at ~0.17% max rel err
                m_t = wk.tile([1, 2], f32, name=f"m_t{tagp}", tag=f"m_t{tagp}")
                nc.vector.tensor_scalar(m_t[:], pss[:], 0.5 / H, 0.5 * EPS,
                                        op0=ALU.mult, op1=ALU.add)
                mi = m_t[:].bitcast(mybir.dt.int32)
                yi_t = wk.tile([1, 2], mybir.dt.int32, name=f"yi{tagp}", tag=f"yi{tagp}")
                nc.vector.tensor_scalar(yi_t[:], mi, 1, None, op0=ALU.logical_shift_right)
                yw = wk.tile([1, 2], f32, name=f"yw{tagp}", tag=f"yw{tagp}")
                t_t = wk.tile([1, 2], f32, name=f"tt{tagp}", tag=f"tt{tagp}")
                nc.vector.tensor_scalar(yw[:].bitcast(mybir.dt.int32), yi_t[:], -1, 0x5EF7520F,
                                        op0=ALU.mult, op1=ALU.add)
                nc.vector.tensor_mul(t_t[:], yw[:], yw[:])
                nc.vector.tensor_mul(t_t[:], t_t[:], m_t[:])
                nc.vector.tensor_scalar(t_t[:], t_t[:], -1.0, 1.5,
                                        op0=ALU.mult, op1=ALU.add)
                yb = wk.tile([1, 2], bf16, name=f"yb{tagp}", tag=f"yb{tagp}")
                nc.vector.tensor_mul(yb[:], yw[:], t_t[:])
                prsb = pp.tile([128, 2], f32, name=f"prsb{tagp}", tag=f"prsb{tagp}")
                nc.tensor.matmul(prsb[:], ones1b[:], yb[:], start=True, stop=True)
                return prsb

            def junction(stage_src_psum, ob_t, resid_t):
                """all-gather partials via ncfw collective; returns x_new [128,8,2] f32.
                stage_src_psum: psum tile [128, 8, 2] partial; ob_t: [128,16] bias or None
                """
                st = wk.tile([128, 16], f32, tag="stg")
                if ob_t is not None:
                    nc.vector.tensor_add(st[:], stage_src_psum[:].rearrange("p t b -> p (t b)"), ob_t)
                else:
                    nc.vector.tensor_copy(st[:], stage_src_psum[:].rearrange("p t b -> p (t b)"))
                jin = dp.tile([128, 16], f32, tag="jin")
                nc.sync.dma_start(out=jin[:], in_=st[:])
                jout = dp.tile([1024, 16], f32, tag="jout", addr_space="Shared")
                nc.gpsimd.collective_compute(
                    "AllGather", ALU.bypass,
                    replica_groups=[list(range(NC))],
                    ins=[jin.opt()], outs=[jout.opt()],
                    unique_tensors="Yes",
                )
                land = wk.tile([128, 8, 16], f32, tag="land")
                nc.sync.dma_start(out=land[:], in_=jout[:, :].rearrange("(r p) f -> p r f", p=128))
                red = wk.tile([128, 16], f32, tag="red")
                nc.vector.tensor_reduce(red[:], land[:].rearrange("p r f -> p f r"),
                                        axis=AX.X, op=ALU.add)
                nx = wk.tile([128, 8, 2], f32, tag="x")
                nc.vector.tensor_add(nx[:].rearrange("p t b -> p (t b)"), red[:],
                                     resid_t[:].rearrange("p t b -> p (t b)"))
                return nx

            att_idx = 0
            for i in range(n_layers):
                bt = BLOCK[i]
                pwT = wp.tile([128, 8, 384], bf16, tag="pwT")
                nc.gpsimd.dma_start(out=pwT[:], in_=din[f'pwT_{i}'][:, :, :])
                small = wp.tile([128, SMALL_W], f32, tag="small")
                nc.gpsimd.dma_start(out=small[:], in_=din[f'small_{i}'][:, :])
                owT = wp.tile([128, 1024], bf16, tag="owT")
                nc.gpsimd.dma_start(out=owT[:], in_=din[f'owT_{i}'][:, :])
                m1T = wp.tile([128, 8, 256], bf16, tag="m1T")
                nc.gpsimd.dma_start(out=m1T[:], in_=din[f'm1T_{i}'][:, :, :])
                m2T = wp.tile([128, 2, 1024], bf16, tag="m2T")
                nc.gpsimd.dma_start(out=m2T[:], in_=din[f'm2T_{i}'][:, :, :])
                if bt == 'ATT':
                    kT = ap_.tile([128, 2, nt2 * 128], bf16, tag="kT")
                    nc.gpsimd.dma_start(out=kT[:], in_=din[f'kT_{att_idx}'][:, :].rearrange(
                        "p (b s) -> p b s", b=2))
                    vv = ap_.tile([128, nt2, 2, 128], bf16, tag="vv")
                    nc.gpsimd.dma_start(out=vv[:], in_=din[f'v_{att_idx}'][:, :].rearrange(
                        "p (t b hd) -> p t b hd", t=nt2, b=2))

                # proj GEMV (n1 + tap2 folded); rsq chain runs in parallel
                pz = pp.tile([128, 3, 2], f32, tag="zh")
                for m in range(3):
                    for kt in range(8):
                        nc.tensor.matmul(pz[:, m, :], pwT[:, kt, m * 128:(m + 1) * 128],
                                         xb[:, kt, :], start=(kt == 0), stop=(kt == 7))
                rsb = rsq_chain(x, "1")

                # zp = pz * rsq + C  (deferred rmsnorm + FIR-1 with host constants)
                zp = wk.tile([128, 3, 2], f32, tag="zp")
                for b in range(2):
                    nc.vector.scalar_tensor_tensor(
                        zp[:, :, b], pz[:, :, b], rsb[:, b:b + 1],
                        small[:, SW_C + b:SW_C + 6:2],
                        op0=ALU.mult, op1=ALU.add)

                if bt != 'ATT':
                    x1v = wk.tile([128, 2], f32, tag="x1v")
                    nc.vector.tensor_mul(x1v[:], zp[:, 1, :], zp[:, 2, :])
                    ytmp = wk.tile([128, 2], f32, tag="ytmp")
                    nc.vector.scalar_tensor_tensor(
                        ytmp[:], x1v[:], small[:, SW_A:SW_A + 1], small[:, SW_E:SW_E + 2],
                        op0=ALU.mult, op1=ALU.add)
                    y2 = wk.tile([128, 2], bf16, tag="y2")
                    nc.vector.tensor_mul(y2[:], ytmp[:], zp[:, 0, :])
                else:
                    # ---- attention ----
                    # rope on q,k jointly: zp[:, 0:2, :] (cols (m=q/k, b))
                    scrA = wk.tile([1, 1], f32, tag="scrA")
                    nc.scalar.activation(scrA[:], zp[0:1, 0, 0:1], AF.Exp)
                    rtmp = wk.tile([128, 2, 2], f32, tag="rtmp")
                    for base in (0, 64):
                        nc.vector.tensor_copy(rtmp[base:base + 32, :, :], zp[base + 32:base + 64, 0:2, :])
                        nc.vector.tensor_copy(rtmp[base + 32:base + 64, :, :], zp[base:base + 32, 0:2, :])
                    qk = wk.tile([128, 2, 2], f32, tag="qk")
                    nc.vector.tensor_scalar_mul(rtmp[:], rtmp[:], ropec[:, 1:2])
                    nc.vector.scalar_tensor_tensor(qk[:], zp[:, 0:2, :], ropec[:, 0:1],
                                                   rtmp[:], op0=ALU.mult, op1=ALU.add)
                    # qbd: block-diag q (bf16); kr: rope'd k
                    nc.vector.tensor_copy(qbd[0:64, :, 0], qk[0:64, 0, :])
                    nc.vector.tensor_copy(qbd[64:128, :, 1], qk[64:128, 0, :])
                    kr = wk.tile([128, 2], bf16, tag="kr")
                    nc.vector.tensor_copy(kr[:], qk[:, 1, :])
                    v_sb = wk.tile([128, 2], f32, tag="v_sb")
                    nc.vector.tensor_copy(v_sb[:], zp[:, 2, :])

                    # scores (transposed): psc [128(s), t, (b,h)]
                    psc = pp.tile([128, nt2, 4], f32, tag="psc")
                    for t_ in range(nt2):
                        pr = 128 if (t_ < nt or rem == 0) else rem
                        if pr < 128:
                            nc.vector.memset(psc[:, t_, :], 0.0)
                        for b in range(2):
                            nc.tensor.matmul(psc[0:pr, t_, 2 * b:2 * b + 2],
                                             kT[:, b, t_ * 128:t_ * 128 + pr],
                                             qbd[:, b, :], start=True, stop=True)
                    # current-token score [1, (b,h)]
                    pcur = pp.tile([1, 2, 2], f32, tag="apsum")
                    for b in range(2):
                        nc.tensor.matmul(pcur[:, b, :], kr[:, b:b + 1], qbd[:, b, :],
                                         start=True, stop=True)
                    # exp
                    esc = wk.tile([128, nt2, 4], bf16, tag="esc")
                    nc.scalar.activation(esc[:], psc[:], AF.Exp)
                    ecur = wk.tile([1, 4], f32, tag="ecur")
                    nc.scalar.activation(ecur[:], pcur[:].rearrange("p b h -> p (b h)"), AF.Exp, scale=SCALE)
                    scrB = wk.tile([1, 1], f32, tag="scrB")
                    nc.scalar.activation(scrB[:], ecur[0:1, 0:1], AF.Gelu_apprx_tanh)
                    # sums over s: ones-matmul -> [1, t*4] -> reduce over t -> [1,4]
                    pse = pp.tile([1, nt2 * 4], f32, name="pse", tag="apsum")
                    nc.tensor.matmul(pse[:], ones128[:], esc[:].rearrange("p t c -> p (t c)"),
                                     start=True, stop=True)
                    sev = wk.tile([1, 8], f32, tag="sev")
                    nc.vector.tensor_reduce(sev[:, 0:4], pse[:].rearrange("p (t c) -> p c t", c=4),
                                            axis=AX.X, op=ALU.add)
                    nc.vector.tensor_add(sev[:, 0:4], sev[:, 0:4], ecur[:])
                    nc.vector.reciprocal(sev[:, 0:4], sev[:, 0:4])
                    nc.vector.tensor_copy(sev[:, 4:8], ecur[:])
                    # broadcast (rec, ecur) to all partitions
                    prcb = pp.tile([128, 8], f32, name="prcb", tag="apsum")
                    nc.tensor.matmul(prcb[:], ones1[:], sev[:], start=True, stop=True)
                    # ctx: py [128, b, hw] accumulated over tiles
                    py = pp.tile([128, 2, 2], f32, tag="py")
                    for b in range(2):
                        for t_ in range(nt2):
                            pr = 128 if (t_ < nt or rem == 0) else rem
                            nc.tensor.matmul(py[:, b, :], vv[0:pr, t_, b, :],
                                             esc[0:pr, t_, 2 * b:2 * b + 2],
                                             start=(t_ == 0), stop=(t_ == nt2 - 1))
                    # y2 = (py_diag + v*ecur_b) * rec_b  per head-half
                    vc_t = wk.tile([128, 2], f32, tag="vc_t")
                    tsum = wk.tile([128, 2], f32, tag="tsum")
                    y2 = wk.tile([128, 2], bf16, tag="y2")
                    for hb, sl in ((0, slice(0, 64)), (1, slice(64, 128))):
                        nc.vector.tensor_mul(vc_t[sl, :], v_sb[sl, :],
                                             prcb[sl, 4 + hb:8:2])
                        nc.vector.tensor_add(tsum[sl, :], py[sl, :, hb], vc_t[sl, :])
                        nc.vector.tensor_mul(y2[sl, :], tsum[sl, :], prcb[sl, hb:4:2])

                # out proj: partial^T [128, 8(t), 2(b)]
                pp1 = pp.tile([128, 8, 2], f32, tag="pj")
                for m in range(8):
                    nc.tensor.matmul(pp1[:, m, :], owT[:, m * 128:(m + 1) * 128], y2[:],
                                     start=True, stop=True)
                x_mid = junction(pp1, small[:, SW_OB:SW_OB + 16], x)

                # mlp (n2 folded into m1T cols; rsq2 applied post-GEMM)
                xmb = wk.tile([128, 8, 2], bf16, tag="xmb")
                nc.vector.tensor_copy(xmb[:], x_mid[:])
                ph = pp.tile([128, 2, 2], f32, name="ph", tag="zh")
                for m in range(2):
                    for kt in range(8):
                        nc.tensor.matmul(ph[:, m, :], m1T[:, kt, m * 128:(m + 1) * 128],
                                         xmb[:, kt, :], start=(kt == 0), stop=(kt == 7))
                rsb2 = rsq_chain(x_mid, "2")
                hz = wk.tile([128, 2, 2], f32, tag="hz")
                for b in range(2):
                    nc.vector.tensor_scalar_mul(hz[:, :, b], ph[:, :, b], rsb2[:, b:b + 1])
                hg = wk.tile([128, 2, 2], bf16, tag="hg")
                nc.scalar.activation(hg[:], hz[:], AF.Gelu_apprx_tanh)
                pp2 = pp.tile([128, 8, 2], f32, name="pp2", tag="pj")
                for m in range(8):
                    for kt in range(2):
                        nc.tensor.matmul(pp2[:, m, :], m2T[:, kt, m * 128:(m + 1) * 128],
                                         hg[:, kt, :], start=(kt == 0), stop=(kt == 1))
                x = junction(pp2, None, x_mid)
                xb = wk.tile([128, 8, 2], bf16, tag="xb")
                nc.vector.tensor_copy(xb[:], x[:])

                if bt == 'ATT':
                    att_idx += 1

            for b in range(2):
                nc.sync.dma_start(out=out_t.ap()[b].rearrange("(t p) -> p t", p=128),
                                  in_=x[:, :, b])

    # patch fake waits -> real remote sems
    for iname, fakenum, snum, sval, sname in patches:
        inst = nc.inst_map[iname]
        ow = inst.sync_info.on_wait
        hits = [w for w in ow if w.id == fakenum]
        assert len(hits) == 1, (iname, [str(w) for w in ow])
        hits[0].id = snum
        hits[0].wait_value = sval
        hits[0].ant_name = sname

    nc.compile()
    return nc


_CACHE = {}


def kernel(**inputs):
    pos = int(np.asarray(inputs['position']))
    if pos not in _CACHE:
        _CACHE[pos] = _build(pos)
    nc = _CACHE[pos]
    in_maps = [_prep_core_inputs(inputs, c, pos) for c in range(NC)]
    res = bass_utils.run_bass_kernel_spmd(nc, in_maps, core_ids=list(range(NC)))
    out = res.results[0]['out']  # [2, 1024], replicated across cores
    return out.reshape(B, 1, H).astype(np.float32)
